# revision 1
# baseline (speedup 1.0000x reference)
"""NeuralOpinionDynamics Trainium2 kernel (8-core SPMD, row-sharded).

out = omega*relu(A_norm @ Z W_D^T) + (1-omega)*softmax(sigmoid(s_i - s_j)) @ Z W_C^T
      + delta*(tanh(Z W1^T + b1) W2^T + b2),   s = Z @ w_V

Sharding: core c owns output rows [1024c, 1024(c+1)). A_norm is symmetric, so
the lhsT-layout column slice A_norm[:, rows_c] equals A_norm[rows_c, :]^T and
is passed per-core. Z^T and the small weights are replicated. No collectives.

Per-core device algorithm (N=8192, D=128, M=1024 rows/core, 64 j-tiles of 128):
 - one fp32r matmul per j-tile computes [Z W_D'^T | Z W_C'^T | -s/2] (N=257)
 - attention tile B[j,i] = exp(sigmoid(s_i-s_j)) via tanh identity:
     X = s_i/2 - s_j/2 (DVE tensor_scalar), t = tanh(X) (ACT),
     B = exp(t/2 + 1/2) (ACT, bf16) -- both funcs in the exp_and_others set
 - dif^T accumulated with fp32r matmuls (zwd stationary, A-slice moving N=512)
 - con (and softmax denominator via an appended ones column, N=129) accumulated
   with bf16 matmuls (B-tile stationary); 2 accumulation groups share each PSUM
   bank (manual start flags: first matmul per bank clears it)
 - reaction MLP + b2 rank-1 matmul; final combine on DVE; dif^T transposed on PE
"""

import sys

sys.path.insert(0, "/opt/trn_rl_repo")

import numpy as np

import concourse.bass as bass
import concourse.mybir as mybir
from concourse import bacc
from concourse.bass_utils import run_bass_kernel_spmd
from concourse.masks import make_identity
from concourse.tile import TileContext

N = 8192
D = 128
NCORES = 8
M = N // NCORES            # rows per core = 1024
JT = N // 128              # j-tiles = 64
IT = M // 128              # i-tiles per core = 8
SLAB = 4                   # j-tiles per ACT slab
NSLAB = JT // SLAB

FP32 = mybir.dt.float32
FP32R = mybir.dt.float32r
BF16 = mybir.dt.bfloat16
AF = mybir.ActivationFunctionType
ALU = mybir.AluOpType

LAST_RESULTS = None


def build_program(reps=1, hwloop=False):
    nc = bacc.Bacc("TRN2", target_bir_lowering=False, debug=False)

    a_cols = nc.dram_tensor("a_cols", [N, M], FP32R, kind="ExternalInput")
    zt_d = nc.dram_tensor("zt", [D, N], FP32R, kind="ExternalInput")
    zi_d = nc.dram_tensor("zi", [D, M], FP32R, kind="ExternalInput")
    rhsw_d = nc.dram_tensor("rhsw", [D, 258], FP32R, kind="ExternalInput")
    wvp_d = nc.dram_tensor("wvp", [D, 2], FP32R, kind="ExternalInput")
    w1t_d = nc.dram_tensor("w1t", [D, D], FP32R, kind="ExternalInput")
    b1_d = nc.dram_tensor("b1", [D, 1], FP32, kind="ExternalInput")
    w2t_d = nc.dram_tensor("w2t", [D, D], FP32, kind="ExternalInput")
    b2_d = nc.dram_tensor("b2", [1, D], FP32, kind="ExternalInput")
    out_d = nc.dram_tensor("out", [M, D], FP32, kind="ExternalOutput")

    with TileContext(nc) as tc:
        with (
            tc.tile_pool(name="persist", bufs=1) as pp,
            tc.tile_pool(name="amove", bufs=3) as ap_pool,
            tc.tile_pool(name="slabs", bufs=2) as sp,
            tc.tile_pool(name="small", bufs=2) as smp,
            tc.tile_pool(name="pz", bufs=2, space="PSUM") as pz,
            tc.tile_pool(name="pdif", bufs=1, space="PSUM") as pdif,
            tc.tile_pool(name="pcon", bufs=1, space="PSUM") as pcon,
        ):
            # ---- persistent SBUF ----
            zt = pp.tile([D, N], FP32R)
            zi = pp.tile([D, M], FP32R)
            rhsw = pp.tile([D, 258], FP32R)
            wvp = pp.tile([D, 2], FP32R)
            w1t = pp.tile([D, D], FP32R)
            b1 = pp.tile([D, 1], FP32)
            w2t = pp.tile([D, D], FP32)
            b2 = pp.tile([1, D], FP32)
            zwc_all = pp.tile([128, JT * 129], BF16)
            h_all = pp.tile([128, JT], FP32)
            sb_bcast = pp.tile([128, M], FP32)
            srow = pp.tile([1, M], FP32)
            ht = pp.tile([D, M], FP32)
            rea_sb = pp.tile([128, M], FP32)
            difrelu = pp.tile([D, M], FP32)
            ones1 = pp.tile([1, 128], FP32)
            half = pp.tile([128, 1], FP32)
            ident = pp.tile([128, 128], FP32)

            nc.sync.dma_start(zi[:], zi_d[:])
            nc.sync.dma_start(rhsw[:], rhsw_d[:])
            nc.sync.dma_start(wvp[:], wvp_d[:])
            nc.sync.dma_start(w1t[:], w1t_d[:])
            nc.sync.dma_start(b1[:], b1_d[:])
            nc.sync.dma_start(w2t[:], w2t_d[:])
            nc.sync.dma_start(b2[:], b2_d[:])
            nc.sync.dma_start(zt[:], zt_d[:])

            nc.vector.memset(ones1[:], 1.0)
            nc.vector.memset(half[:], 0.5)
            make_identity(nc, ident[:])
            # ones column (bf16 1.0) for the softmax denominator, all j-tiles
            zwc_view = zwc_all[:].rearrange("p (t c) -> p t c", c=129)
            nc.vector.memset(zwc_view[:, :, 128], 1.0)

            # ---- PSUM accumulators ----
            ps_dif = pdif.tile([D, M], FP32)          # dif^T, 2 banks
            ps_con = [
                pcon.tile([128, 258], FP32, tag=f"con{b}", name=f"ps_con{b}")
                for b in range(4)
            ]

            import contextlib
            rep_ctx = tc.For_i(0, reps, 1) if hwloop and reps > 1 else None
            for _rep in range(1 if hwloop else reps):
              with (rep_ctx if rep_ctx is not None else contextlib.nullcontext()):
                # ---- startup: s row, its broadcast, reaction MLP ----
                # srow[0, i] = 0.5 * s_i for this core's rows
                for hh in range(2):
                    srow_ps = pz.tile([2, 512], FP32, tag="zw")
                    nc.tensor.matmul(
                        srow_ps[:],
                        wvp[:],
                        zi[:, hh * 512 : (hh + 1) * 512],
                        start=True, stop=True, skip_group_check=True,
                    )
                    nc.vector.tensor_copy(srow[:, hh * 512 : (hh + 1) * 512], srow_ps[0:1, :])

                for hh in range(2):
                    sbb = pz.tile([128, 512], FP32, tag="zw")
                    nc.tensor.matmul(
                        sbb[:], ones1[:], srow[:, hh * 512 : (hh + 1) * 512],
                        start=True, stop=True, skip_group_check=True,
                    )
                    nc.vector.tensor_copy(sb_bcast[:, hh * 512 : (hh + 1) * 512], sbb[:])

                for hh in range(2):
                    t1 = pz.tile([D, 512], FP32, tag="zw")
                    nc.tensor.matmul(
                        t1[:], w1t[:], zi[:, hh * 512 : (hh + 1) * 512],
                        start=True, stop=True, skip_group_check=True,
                    )
                    nc.scalar.activation(
                        ht[:, hh * 512 : (hh + 1) * 512], t1[:], AF.Tanh,
                        bias=b1[:], scale=1.0,
                    )

                # rea groups: 4 accumulation groups of [128,128] per PSUM bank tile
                for half_i in range(2):
                    rea_ps = pz.tile([128, 512], FP32, tag="zw")
                    for q in range(4):
                        it = half_i * 4 + q
                        sl = slice(q * 128, (q + 1) * 128)
                        nc.tensor.matmul(
                            rea_ps[:, sl], ht[:, it * 128 : (it + 1) * 128], w2t[:],
                            start=(q == 0), stop=False, skip_group_check=True,
                        )
                        nc.tensor.matmul(
                            rea_ps[:, sl], ones1[:], b2[:],
                            start=False, stop=(q == 3), skip_group_check=True,
                        )
                    nc.vector.tensor_copy(
                        rea_sb[:, half_i * 512 : (half_i + 1) * 512], rea_ps[:]
                    )

                # ---- main loop over ACT slabs of SLAB j-tiles ----
                for g in range(NSLAB):
                    x_slab = sp.tile([128, SLAB * M], FP32, tag="x")
                    for q in range(SLAB):
                        jt = g * SLAB + q
                        asb = ap_pool.tile([128, M], FP32R, tag="a")
                        nc.sync.dma_start(asb[:], a_cols[jt * 128 : (jt + 1) * 128, :])

                        # fused small matmul: [zwd | zwc | -s/2] for this j-tile
                        zw_ps = pz.tile([128, 258], FP32, tag="zw")
                        nc.tensor.matmul(
                            zw_ps[:], zt[:, jt * 128 : (jt + 1) * 128], rhsw[:],
                            start=True, stop=True, skip_group_check=True,
                        )
                        zwd = smp.tile([128, 128], FP32R, tag="zwd")
                        nc.vector.tensor_copy(zwd[:], zw_ps[:, 0:128])
                        nc.vector.tensor_copy(
                            zwc_all[:, jt * 129 : jt * 129 + 128], zw_ps[:, 128:256]
                        )
                        nc.vector.tensor_copy(h_all[:, jt : jt + 1], zw_ps[:, 256:257])

                        # X[p, i] = 0.5*s_i - 0.5*s_jp
                        nc.vector.tensor_scalar(
                            x_slab[:, q * M : (q + 1) * M], sb_bcast[:],
                            h_all[:, jt : jt + 1], None, op0=ALU.add,
                        )

                        # dif^T += zwd^T @ A-slice   (fp32r, N=512)
                        for hh in range(2):
                            nc.tensor.matmul(
                                ps_dif[:, hh * 512 : (hh + 1) * 512],
                                zwd[:],
                                asb[:, hh * 512 : (hh + 1) * 512],
                                start=(jt == 0), stop=(jt == JT - 1),
                                skip_group_check=True,
                            )

                    # B = exp(0.5*tanh(X) + 0.5) = exp(sigmoid(2X)), bf16
                    t_slab = sp.tile([128, SLAB * M], BF16, tag="t")
                    nc.scalar.activation(t_slab[:], x_slab[:], AF.Tanh, bias=0.0, scale=1.0)
                    b_slab = sp.tile([128, SLAB * M], BF16, tag="b")
                    nc.scalar.activation(
                        b_slab[:], t_slab[:], AF.Exp, bias=half[:], scale=0.5
                    )

                    # con += B^T-tile @ [zwc | 1]  (bf16, N=129, 2 groups per bank)
                    for q in range(SLAB):
                        jt = g * SLAB + q
                        for it in range(IT):
                            bank, off = it // 2, (it % 2) * 129
                            nc.tensor.matmul(
                                ps_con[bank][:, off : off + 129],
                                b_slab[:, q * M + it * 128 : q * M + (it + 1) * 128],
                                zwc_all[:, jt * 129 : (jt + 1) * 129],
                                start=(jt == 0 and it % 2 == 0),
                                stop=(jt == JT - 1 and it % 2 == 1),
                                skip_group_check=True,
                            )

                # ---- finish: relu, transpose dif^T, combine, write out ----
                nc.vector.tensor_scalar(difrelu[:], ps_dif[:], 0.0, None, op0=ALU.max)

                for it in range(IT):
                    dift = pz.tile([128, 128], FP32, tag="zw")
                    nc.tensor.transpose(
                        dift[:], difrelu[:, it * 128 : (it + 1) * 128], ident[:]
                    )
                    bank, off = it // 2, (it % 2) * 129
                    rcp = smp.tile([128, 1], FP32, tag="rcp")
                    nc.vector.reciprocal(rcp[:], ps_con[bank][:, off + 128 : off + 129])
                    o1 = smp.tile([128, 128], FP32, tag="o1")
                    nc.vector.tensor_scalar(
                        o1[:], ps_con[bank][:, off : off + 128], rcp[:], None,
                        op0=ALU.mult,
                    )
                    o2 = smp.tile([128, 128], FP32, tag="o2")
                    nc.vector.tensor_add(o2[:], o1[:], dift[:])
                    o3 = smp.tile([128, 128], FP32, tag="o3")
                    nc.vector.tensor_add(
                        o3[:], o2[:], rea_sb[:, it * 128 : (it + 1) * 128]
                    )
                    nc.sync.dma_start(out_d[it * 128 : (it + 1) * 128, :], o3[:])

    nc.compile()
    return nc


def kernel(Z, A_norm, W_D, W_C, w_V, W1, b1, W2, b2, omega_logit, delta_logit):
    global LAST_RESULTS
    Z = np.asarray(Z, dtype=np.float32)
    A_norm = np.asarray(A_norm, dtype=np.float32)
    omega = float(1.0 / (1.0 + np.exp(-np.float32(omega_logit))))
    delta = float(1.0 / (1.0 + np.exp(-np.float32(delta_logit))))

    # host-side layout prep (scalar folding + transposes only)
    wd_s = (omega * np.asarray(W_D, np.float32)).T            # [D, D] = W_D'^T
    wc_s = ((1.0 - omega) * np.asarray(W_C, np.float32)).T    # [D, D] = W_C'^T
    wv = np.asarray(w_V, np.float32).reshape(D, 1)
    rhsw = np.concatenate(
        [wd_s, wc_s, -0.5 * wv, np.zeros((D, 1), np.float32)], axis=1
    )                                                         # [D, 258]
    rhsw = np.ascontiguousarray(rhsw, np.float32)
    zt = np.ascontiguousarray(Z.T)                            # [D, N]
    w1t = np.ascontiguousarray(np.asarray(W1, np.float32).T)  # [D, D]
    w2t = np.ascontiguousarray((delta * np.asarray(W2, np.float32)).T)
    b1c = np.ascontiguousarray(np.asarray(b1, np.float32).reshape(D, 1))
    b2r = np.ascontiguousarray((delta * np.asarray(b2, np.float32)).reshape(1, D))
    wvp = np.ascontiguousarray(np.concatenate([0.5 * wv, np.zeros((D, 1), np.float32)], axis=1))

    shared = {
        "zt": zt, "rhsw": rhsw, "wvp": wvp, "w1t": w1t,
        "b1": b1c, "w2t": w2t, "b2": b2r,
    }
    in_maps = []
    for c in range(NCORES):
        sl = slice(c * M, (c + 1) * M)
        in_maps.append({
            **shared,
            "a_cols": np.ascontiguousarray(A_norm[:, sl]),
            "zi": np.ascontiguousarray(zt[:, sl]),
        })

    nc = build_program()
    LAST_RESULTS = run_bass_kernel_spmd(nc, in_maps, list(range(NCORES)))
    return np.concatenate(
        [LAST_RESULTS.results[c]["out"] for c in range(NCORES)], axis=0
    )

